# revision 1
# baseline (speedup 1.0000x reference)
"""Trainium2 Bass kernel for the KG triple-scoring head (nn_ClassifierHead).

score[t] = emb[s_t]·Ws[r_t] + emb[o_t]·Wo[r_t] + b[r_t]

Strategy (8-core SPMD, data-parallel over nodes + owner-routed requests):
  * The per-relation weight table is tiny (128 projection vectors of dim 512),
    so instead of gathering 2KB embedding rows per triple we precompute
    T[j, n] = sum_d W2T[d, j] * embT[d, n]  (the projection of every node onto
    all 128 relation half-vectors) with dense PE matmuls — nodes sharded
    12500/core.
  * Each triple contributes two scalar lookups T[col, node] (col = r for the
    subject half, 64+r for the object half). Lookups are routed to the core
    that owns the node. On-device the lookup is done with a one-hot matmul
    (Z = OHcol^T @ T_block) followed by a one-hot row-select on DVE
    (V = reduce_n(Z * OHrow)) — no indirect DMA, no transposes.
  * Host combines the two halves and adds b[r].
"""
import numpy as np
import ml_dtypes

N_NODES = 100000
N_DIM = 512
N_REL = 64
N_TRIPLES = 200000
NCORES = 8
NSHARD = N_NODES // NCORES   # 12500
NPAD = 12544                 # 98 * 128 (zero-padded so every block is full)
NBLK = NPAD // 128           # 98
TPB = 5                      # request tiles per 128-node block
NT = 492                     # 98*5 real tiles + 2 dummy, divisible by 4
NBATCH = NT // 4

_CACHE = {}


def _build_kernel():
    import concourse.bacc as bacc
    import concourse.mybir as mybir
    from concourse.tile import TileContext

    BF16 = mybir.dt.bfloat16
    F32 = mybir.dt.float32

    nc = bacc.Bacc("TRN2", target_bir_lowering=False, debug=False, num_devices=NCORES)
    embT = nc.dram_tensor("embT", [N_DIM, NPAD], BF16, kind="ExternalInput").ap()
    w2t = nc.dram_tensor("w2t", [N_DIM, 128], BF16, kind="ExternalInput").ap()
    ohcol = nc.dram_tensor("ohcol", [128, NT * 128], BF16, kind="ExternalInput").ap()
    rowrel = nc.dram_tensor("rowrel", [128, NT], BF16, kind="ExternalInput").ap()
    iota = nc.dram_tensor("iota", [128, 512], BF16, kind="ExternalInput").ap()
    vout = nc.dram_tensor("vout", [128, NT], F32, kind="ExternalOutput").ap()

    with TileContext(nc) as tc:
        with (
            tc.tile_pool(name="const", bufs=1) as cpool,
            tc.tile_pool(name="et", bufs=3) as etpool,
            tc.tile_pool(name="tt", bufs=1) as tpool,
            tc.tile_pool(name="oh", bufs=4) as ohpool,
            tc.tile_pool(name="ps", bufs=3, space="PSUM") as pspool,
            tc.tile_pool(name="out", bufs=1) as outpool,
        ):
            w2t_sb = []
            for d in range(4):
                w = cpool.tile([128, 128], BF16, tag=f"w{d}", name=f"w{d}")
                nc.sync.dma_start(out=w[:], in_=w2t[d * 128:(d + 1) * 128, :])
                w2t_sb.append(w)
            iota_sb = cpool.tile([128, 512], BF16, tag="iota")
            nc.sync.dma_start(out=iota_sb[:], in_=iota[:])
            rr_sb = cpool.tile([128, NT], BF16, tag="rr")
            nc.sync.dma_start(out=rr_sb[:], in_=rowrel[:])
            vout_sb = outpool.tile([128, NT], F32, tag="vout")

            T_sb = [tpool.tile([128, 512], BF16, tag=f"T{c}", name=f"T{c}") for c in range(25)]

            # ---- phase A: T[j, n] = W2T^T @ embT ----
            for ci, c0 in enumerate(range(0, NPAD, 512)):
                nw = min(512, NPAD - c0)
                et = []
                for d in range(4):
                    t = etpool.tile([128, 512], BF16, tag=f"et{d}")
                    nc.sync.dma_start(out=t[:, :nw], in_=embT[d * 128:(d + 1) * 128, c0:c0 + nw])
                    et.append(t)
                psA = pspool.tile([128, 512], F32, tag="psA")
                for d in range(4):
                    nc.tensor.matmul(out=psA[:, :nw], lhsT=w2t_sb[d][:], rhs=et[d][:, :nw],
                                     start=(d == 0), stop=(d == 3))
                nc.vector.tensor_copy(out=T_sb[ci][:, :nw], in_=psA[:, :nw])

            # ---- phase B: per request tile, Z = OHcol^T @ T_block; V = Σ_n Z⊙OHrow ----
            for bt in range(NBATCH):
                oh_sb = ohpool.tile([128, 512], BF16, tag="ohcol")
                nc.sync.dma_start(out=oh_sb[:], in_=ohcol[:, bt * 512:(bt + 1) * 512])
                psZ = pspool.tile([128, 512], mybir.dt.float32, tag="psZ")
                for q in range(4):
                    t = 4 * bt + q
                    blk = min(t // TPB, NBLK - 1)
                    ci, sub = blk // 4, blk % 4
                    nc.tensor.matmul(out=psZ[:, q * 128:(q + 1) * 128],
                                     lhsT=oh_sb[:, q * 128:(q + 1) * 128],
                                     rhs=T_sb[ci][:, sub * 128:(sub + 1) * 128],
                                     start=True, stop=True)
                ohrow = ohpool.tile([128, 512], BF16, tag="ohrow")
                rr3 = rr_sb[:, bt * 4:(bt + 1) * 4].to_broadcast([128, 4, 128])
                nc.vector.tensor_tensor(out=ohrow[:].rearrange("p (g x) -> p g x", x=128),
                                        in0=rr3, in1=iota_sb[:].rearrange("p (g x) -> p g x", x=128),
                                        op=mybir.AluOpType.is_equal)
                msel = ohpool.tile([128, 512], BF16, tag="msel")
                nc.vector.tensor_tensor(out=msel[:], in0=psZ[:], in1=ohrow[:],
                                        op=mybir.AluOpType.mult)
                nc.vector.tensor_reduce(out=vout_sb[:, bt * 4:(bt + 1) * 4],
                                        in_=msel[:].rearrange("p (g x) -> p g x", x=128),
                                        axis=mybir.AxisListType.X, op=mybir.AluOpType.add)
            nc.sync.dma_start(out=vout[:], in_=vout_sb[:])
    nc.compile()
    return nc


class _SpmdRunner:
    """Executes the compiled Bass module on the 8 NeuronCores via PJRT."""

    def __init__(self, nc):
        import jax
        import concourse.mybir as mybir
        from jax.sharding import Mesh, PartitionSpec
        from jax.experimental.shard_map import shard_map
        from concourse.bass2jax import _bass_exec_p, partition_id_tensor, install_neuronx_cc_hook

        install_neuronx_cc_hook()
        self.jax = jax
        partition_name = nc.partition_id_tensor.name if nc.partition_id_tensor else None
        in_names, out_names, out_avals = [], [], []
        for alloc in nc.m.functions[0].allocations:
            if not isinstance(alloc, mybir.MemoryLocationSet):
                continue
            name = alloc.memorylocations[0].name
            if alloc.kind == "ExternalInput":
                if name != partition_name:
                    in_names.append(name)
            elif alloc.kind == "ExternalOutput":
                out_names.append(name)
                out_avals.append(jax.core.ShapedArray(tuple(alloc.tensor_shape),
                                                      mybir.dt.np(alloc.dtype)))
        self.in_names, self.out_names, self.out_avals = in_names, out_names, out_avals
        n_params = len(in_names)
        all_in = list(in_names) + list(out_names)
        if partition_name is not None:
            all_in.append(partition_name)

        def _body(*args):
            operands = list(args)
            if partition_name is not None:
                operands.append(partition_id_tensor())
            return tuple(_bass_exec_p.bind(
                *operands, out_avals=tuple(out_avals), in_names=tuple(all_in),
                out_names=tuple(out_names), lowering_input_output_aliases=(),
                sim_require_finite=True, sim_require_nnan=True, nc=nc))

        devices = jax.devices()[:NCORES]
        mesh = Mesh(np.asarray(devices), ("core",))
        in_specs = (PartitionSpec("core"),) * (n_params + len(out_names))
        out_specs = (PartitionSpec("core"),) * len(out_names)
        self.fn = jax.jit(shard_map(_body, mesh=mesh, in_specs=in_specs,
                                    out_specs=out_specs, check_rep=False),
                          keep_unused=True)
        self.sharding = jax.sharding.NamedSharding(mesh, PartitionSpec("core"))

    def run(self, in_maps):
        jax = self.jax
        args = []
        for name in self.in_names:
            cat = np.concatenate([np.asarray(in_maps[c][name]) for c in range(NCORES)], axis=0)
            args.append(jax.device_put(cat, self.sharding))
        for av in self.out_avals:
            z = np.zeros((NCORES * av.shape[0], *av.shape[1:]), av.dtype)
            args.append(jax.device_put(z, self.sharding))
        outs = [np.asarray(o) for o in self.fn(*args)]
        return [
            {name: outs[i].reshape(NCORES, *self.out_avals[i].shape)[c]
             for i, name in enumerate(self.out_names)}
            for c in range(NCORES)
        ]


def _route_requests(triples):
    t = np.asarray(triples).astype(np.int64)
    s, r, o = t[0], t[1], t[2]
    gidx = np.concatenate([s, o])
    col = np.concatenate([r, 64 + r]).astype(np.int64)
    owner = gidx // NSHARD
    lrow = gidx % NSHARD
    blk = lrow // 128
    rowr = lrow % 128
    key = owner * NBLK + blk
    order = np.argsort(key, kind="stable")
    counts = np.bincount(key, minlength=NCORES * NBLK).reshape(NCORES, NBLK)
    assert counts.max() <= TPB * 128, counts.max()
    per_core = []
    pos = 0
    for k in range(NCORES):
        oh = np.zeros((128, NT * 128), np.float32)
        rr = np.zeros((128, NT), np.float32)
        ids = np.full((128, NT), -1, np.int64)
        for b in range(NBLK):
            n = counts[k, b]
            if n == 0:
                continue
            req = order[pos:pos + n]
            pos += n
            t0 = b * TPB
            tt = t0 + np.arange(n) // 128
            ii = np.arange(n) % 128
            oh[col[req], tt * 128 + ii] = 1.0
            rr[ii, tt] = rowr[req]
            ids[ii, tt] = req
        per_core.append((oh.astype(ml_dtypes.bfloat16), rr.astype(ml_dtypes.bfloat16), ids))
    assert pos == 2 * N_TRIPLES
    return per_core


def kernel(embeddings, W, b, triples):
    emb = np.asarray(embeddings, dtype=np.float32)
    Wn = np.asarray(W, dtype=np.float32)
    bn = np.asarray(b, dtype=np.float32)
    tr = np.asarray(triples)

    W2T = np.concatenate([Wn[:, :N_DIM], Wn[:, N_DIM:]], axis=0).T  # [512, 128]
    w2t_bf = np.ascontiguousarray(W2T).astype(ml_dtypes.bfloat16)
    embT = emb.T.astype(ml_dtypes.bfloat16)
    iota = np.tile(np.arange(128, dtype=np.float32), (128, 4)).astype(ml_dtypes.bfloat16)
    per_core = _route_requests(tr)

    in_maps, ids_list = [], []
    for k in range(NCORES):
        oh, rr, ids = per_core[k]
        ek = np.zeros((N_DIM, NPAD), ml_dtypes.bfloat16)
        ek[:, :NSHARD] = embT[:, k * NSHARD:(k + 1) * NSHARD]
        in_maps.append({"embT": ek, "w2t": w2t_bf, "ohcol": oh, "rowrel": rr, "iota": iota})
        ids_list.append(ids)

    if "runner" not in _CACHE:
        _CACHE["runner"] = _SpmdRunner(_build_kernel())
    results = _CACHE["runner"].run(in_maps)

    r = tr[1].astype(np.int64)
    allvals = np.empty(2 * N_TRIPLES, np.float32)
    for k in range(NCORES):
        v = results[k]["vout"]
        ids = ids_list[k]
        m = ids >= 0
        allvals[ids[m]] = v[m]
    return (allvals[:N_TRIPLES] + allvals[N_TRIPLES:] + bn[r]).astype(np.float32)
